# revision 14
# baseline (speedup 1.0000x reference)
"""Multi-head attention (B=2, S=4096, D=512, H=8) on 8 Trainium2 NeuronCores.

Sharding: batch x head-pair.  Core c handles batch b = c//4 and heads
(2*(c%4), 2*(c%4)+1).  Each core computes its heads' Q/K/V projections,
flash-style attention (scores kept transposed [kv, q] so the attn@V matmul
consumes the exp() output directly, with softmax denominators accumulated via
an extra ones-column on V), and its heads' slice of the output projection.
The 4 per-batch partial outputs are summed on the host (row-parallel linear)
and the output bias is added there.

All matmul operands are bf16 (f32 PSUM accumulation); inputs are transposed
and cast on the host so the device consumes [d, s]-layout activations
directly (no on-device transposes).
"""

import sys

sys.path.insert(0, "/opt/trn_rl_repo")

import numpy as np
import ml_dtypes

import concourse.bacc as bacc
import concourse.bass as bass
import concourse.tile as tile
from concourse import mybir
from concourse.bass_utils import run_bass_kernel_spmd

BF16 = ml_dtypes.bfloat16

B = 2
S = 4096
D = 512
H = 8
DH = 64           # head dim
HPC = 2           # heads per core
D2 = HPC * DH     # 128, the two heads' feature slice
N_CORES = 8
QC = 512          # query chunk (free dim of scores/attnV matmuls)
KVC = 128         # kv chunk (partition dim of transposed scores)
N_QC = S // QC    # 8
N_KVC = S // KVC  # 32
GROUP = 3         # kv chunks per exp() instruction (PSUM banks per S tile)

FP32 = mybir.dt.float32
BF16_T = mybir.dt.bfloat16
AF = mybir.ActivationFunctionType


def build_kernel():
    nc = bacc.Bacc("TRN2", debug=False, enable_asserts=False, num_devices=N_CORES)

    # DRAM I/O (per-core shapes; identical program on every core)
    qT = nc.dram_tensor("qT", [D, S], BF16_T, kind="ExternalInput").ap()
    kT = nc.dram_tensor("kT", [D, S], BF16_T, kind="ExternalInput").ap()
    vT = nc.dram_tensor("vT", [D, S], BF16_T, kind="ExternalInput").ap()
    wqT2 = nc.dram_tensor("wqT2", [D, D2], BF16_T, kind="ExternalInput").ap()
    wkT2 = nc.dram_tensor("wkT2", [D, D2], BF16_T, kind="ExternalInput").ap()
    wvT2 = nc.dram_tensor("wvT2", [D, D2], BF16_T, kind="ExternalInput").ap()
    wo0 = nc.dram_tensor("wo0", [DH, D], BF16_T, kind="ExternalInput").ap()
    wo1 = nc.dram_tensor("wo1", [DH, D], BF16_T, kind="ExternalInput").ap()
    bq2 = nc.dram_tensor("bq2", [D2, 1], FP32, kind="ExternalInput").ap()
    bk2 = nc.dram_tensor("bk2", [D2, 1], FP32, kind="ExternalInput").ap()
    bv2 = nc.dram_tensor("bv2", [1, D2], BF16_T, kind="ExternalInput").ap()
    ouT = nc.dram_tensor("ouT", [D, S], FP32, kind="ExternalOutput").ap()

    KD = D // 128  # 4 contraction chunks of 128

    with tile.TileContext(nc) as tc:
        with (
            tc.tile_pool(name="persist", bufs=1) as pp,
            tc.tile_pool(name="stream", bufs=6) as ps,
            tc.tile_pool(name="ptpool", bufs=3) as ppt,
            tc.tile_pool(name="norm", bufs=2) as pn,
            tc.tile_pool(name="outs", bufs=4) as po,
            tc.tile_pool(name="psum", bufs=1, space="PSUM") as psum,
        ):
            # ---- constants / weights to SBUF ----
            wq_sb = pp.tile([128, KD, D2], BF16_T)
            wk_sb = pp.tile([128, KD, D2], BF16_T)
            wv_sb = pp.tile([128, KD, D2], BF16_T)
            nc.sync.dma_start(out=wq_sb, in_=wqT2.rearrange("(c p) m -> p c m", p=128))
            nc.sync.dma_start(out=wk_sb, in_=wkT2.rearrange("(c p) m -> p c m", p=128))
            nc.sync.dma_start(out=wv_sb, in_=wvT2.rearrange("(c p) m -> p c m", p=128))
            wo_sb = [pp.tile([DH, D], BF16_T, tag=f"wo{h}", name=f"wo{h}") for h in range(HPC)]
            nc.sync.dma_start(out=wo_sb[0], in_=wo0)
            nc.sync.dma_start(out=wo_sb[1], in_=wo1)
            bq_sb = pp.tile([D2, 1], FP32, tag="bq")
            bk_sb = pp.tile([D2, 1], FP32, tag="bk")
            bv_sb = pp.tile([1, D2], BF16_T, tag="bv")
            nc.sync.dma_start(out=bq_sb, in_=bq2)
            nc.sync.dma_start(out=bk_sb, in_=bk2)
            nc.sync.dma_start(out=bv_sb, in_=bv2)
            ones_row = pp.tile([1, 128], BF16_T, tag="ones")
            nc.vector.memset(ones_row, 1.0)

            # ---- persistent activations ----
            qpT = pp.tile([D2, S], BF16_T, tag="qpT")   # [2*dh, s] both heads
            kpT = pp.tile([D2, S], BF16_T, tag="kpT")
            # vp per head: [kv in chunk, chunk, dh+1]; last col = ones (denom)
            vp = [pp.tile([128, N_KVC, 128], BF16_T, tag=f"vp{h}", name=f"vp{h}") for h in range(HPC)]
            for h in range(HPC):
                nc.vector.memset(vp[h][:, :, DH + 1 :], 0.0)
                nc.vector.memset(vp[h][:, :, DH : DH + 1], 1.0)

            # ---- stage A: projections ----
            # vp first: the flash loop's attnV needs it; q/k projections
            # overlap the first flash groups.
            # vp: out[s, j2] = x @ W.T  (lhsT = vT chunk (stationary), rhs = w chunk)
            vin = [ps.tile([128, S], BF16_T, tag="xin", name=f"vin_{kc}") for kc in range(KD)]
            for kc in range(KD):
                nc.sync.dma_start(out=vin[kc], in_=vT[kc * 128 : (kc + 1) * 128, :])
            for sc in range(N_KVC):
                pt = psum.tile([128, D2], FP32, tag=f"o{sc % 2}", name="pt_v")
                for kc in range(KD):
                    nc.tensor.matmul(
                        pt,
                        vin[kc][:, sc * 128 : (sc + 1) * 128],
                        wv_sb[:, kc, :],
                        start=(kc == 0),
                        stop=False,
                    )
                # bias via rank-1 update: ones[s] x bv[j2]
                nc.tensor.matmul(pt, ones_row, bv_sb, start=False, stop=True)
                for h in range(HPC):
                    nc.vector.tensor_copy(
                        out=vp[h][:, sc, 0:DH], in_=pt[:, h * DH : (h + 1) * DH]
                    )
            # qpT / kpT: out[j2, s] = W.T @ xT  (lhsT = w chunk, rhs = xT chunk)
            for name, src_, w_sb, b_sb, dst in (
                ("q", qT, wq_sb, bq_sb, qpT),
                ("k", kT, wk_sb, bk_sb, kpT),
            ):
                xin = [ps.tile([128, S], BF16_T, tag="xin", name=f"xin_{name}_{kc}") for kc in range(KD)]
                for kc in range(KD):
                    nc.sync.dma_start(out=xin[kc], in_=src_[kc * 128 : (kc + 1) * 128, :])
                for sc in range(S // 512):
                    pt = psum.tile([D2, 512], FP32, tag=f"s{sc % 2}", name="pt_qk")
                    for kc in range(KD):
                        nc.tensor.matmul(
                            pt,
                            w_sb[:, kc, :],
                            xin[kc][:, sc * 512 : (sc + 1) * 512],
                            start=(kc == 0),
                            stop=(kc == KD - 1),
                        )
                    # evict + per-partition bias on DVE (keeps ACT free for exp)
                    nc.vector.tensor_scalar_add(
                        out=dst[:, sc * 512 : (sc + 1) * 512],
                        in0=pt,
                        scalar1=b_sb,
                    )
            # ---- stage B: attention + output projection ----
            # per-head kv chunk groups: h0 uses 3-bank groups, h1 2-bank
            # (3+2+1+1 attn banks + 1 dedicated outproj bank = 7 of 8)
            def make_groups(g):
                out, kv = [], 0
                while kv < N_KVC:
                    n = min(g, N_KVC - kv)
                    out.append((kv, n))
                    kv += n
                return out
            hgroups = [make_groups(3), make_groups(2)]

            events = sorted(
                [(g0, h, glen) for h in range(HPC) for g0, glen in hgroups[h]]
            )

            o_tiles = {}

            def make_attn(qc, g0, h, glen, p_sb):
                def emit():
                    for gi in range(glen):
                        kvc = g0 + gi
                        nc.tensor.matmul(
                            o_tiles[(qc, h)],
                            vp[h][:, kvc, :],
                            p_sb[:, gi * QC : (gi + 1) * QC],
                            start=(kvc == 0),
                            stop=(kvc == N_KVC - 1),
                        )
                return emit

            def make_norm(qc):
                def emit():
                    ous = []
                    den2 = pn.tile([1, HPC * QC], FP32, tag="den2", name="den2")
                    for h in range(HPC):
                        ou = pn.tile([DH, QC], FP32, tag=f"ou{h}", name=f"ou{h}")
                        nc.vector.tensor_copy(out=ou, in_=o_tiles[(qc, h)][0:DH, :])
                        nc.vector.tensor_copy(
                            out=den2[0:1, h * QC : (h + 1) * QC],
                            in_=o_tiles[(qc, h)][DH : DH + 1, :],
                        )
                        ous.append(ou)
                    rec2 = pn.tile([1, HPC * QC], FP32, tag="rec2", name="rec2")
                    nc.vector.reciprocal_approx_fast(out=rec2, in_=den2)
                    outn = []
                    for h in range(HPC):
                        bcast = pn.tile([DH, QC], FP32, tag=f"bcast{h}", name=f"bcast{h}")
                        nc.gpsimd.partition_broadcast(
                            bcast, rec2[0:1, h * QC : (h + 1) * QC]
                        )
                        on = pn.tile([DH, QC], BF16_T, tag=f"outn{h}", name=f"on{h}")
                        nc.vector.tensor_mul(on, ous[h], bcast)
                        outn.append(on)
                    return outn
                return emit

            def make_proj(qc, outn):
                def emit():
                    qs = slice(qc * QC, (qc + 1) * QC)
                    for ec in range(D // 128):
                        op = psum.tile([128, QC], FP32, tag="op", name="op")
                        nc.tensor.matmul(
                            op, wo_sb[0][:, ec * 128 : (ec + 1) * 128], outn[0],
                            start=True, stop=False,
                        )
                        nc.tensor.matmul(
                            op, wo_sb[1][:, ec * 128 : (ec + 1) * 128], outn[1],
                            start=False, stop=True,
                        )
                        ot = po.tile([128, QC], FP32, tag="ot", name="ot")
                        nc.vector.tensor_copy(out=ot, in_=op)
                        nc.sync.dma_start(
                            out=ouT[ec * 128 : (ec + 1) * 128, qs], in_=ot
                        )
                return emit

            # software pipeline: attnV of each event is deferred one event so
            # the PE never sits behind the exp() it just fed; per-qc epilogue
            # (normalize + outproj) is deferred past that attnV.
            prev_attn = None
            pending_norm = None
            pending_proj = None
            for qc in range(N_QC):
                qs = slice(qc * QC, (qc + 1) * QC)
                for h in range(HPC):
                    o_tiles[(qc, h)] = psum.tile(
                        [128, QC], FP32, tag=f"o{h}", name=f"o_ps{h}"
                    )
                for evi, (g0, h, glen) in enumerate(events):
                    hs = slice(h * DH, (h + 1) * DH)
                    s_ps = psum.tile([128, glen * QC], FP32, tag=f"s{h}", name=f"s_ps{h}")
                    for gi in range(glen):
                        kvc = g0 + gi
                        nc.tensor.matmul(
                            s_ps[:, gi * QC : (gi + 1) * QC],
                            kpT[hs, kvc * KVC : (kvc + 1) * KVC],
                            qpT[hs, qs],
                            start=True,
                            stop=True,
                        )
                    p_sb = ppt.tile([128, glen * QC], BF16_T, tag=f"pt{h}", name=f"p_sb{h}")
                    nc.scalar.activation(
                        out=p_sb, in_=s_ps, func=AF.Exp, scale=0.125
                    )
                    if prev_attn is not None:
                        prev_attn()
                    prev_attn = make_attn(qc, g0, h, glen, p_sb)
                    if pending_norm is not None:
                        outn = pending_norm()
                        pending_proj = make_proj(qc - 1, outn)
                        pending_norm = None
                    elif pending_proj is not None and evi >= 6:
                        pending_proj()
                        pending_proj = None
                pending_norm = make_norm(qc)
            prev_attn()
            outn = pending_norm()
            make_proj(N_QC - 1, outn)()
    nc.compile()
    return nc


_NC_CACHE = None


def _get_nc():
    global _NC_CACHE
    if _NC_CACHE is None:
        _NC_CACHE = build_kernel()
    return _NC_CACHE


def make_in_maps(q, k, v, w_q, b_q, w_k, b_k, w_v, b_v, w_o, b_o):
    """Shard the full inputs into the 8 per-core input maps."""
    q = np.asarray(q, np.float32)
    k = np.asarray(k, np.float32)
    v = np.asarray(v, np.float32)
    w_q = np.asarray(w_q, np.float32)
    w_k = np.asarray(w_k, np.float32)
    w_v = np.asarray(w_v, np.float32)
    w_o = np.asarray(w_o, np.float32)
    b_q = np.asarray(b_q, np.float32)
    b_k = np.asarray(b_k, np.float32)
    b_v = np.asarray(b_v, np.float32)

    qT = [np.ascontiguousarray(q[b].T).astype(BF16) for b in range(B)]
    kTb = [np.ascontiguousarray(k[b].T).astype(BF16) for b in range(B)]
    vTb = [np.ascontiguousarray(v[b].T).astype(BF16) for b in range(B)]
    wqT = np.ascontiguousarray(w_q.T).astype(BF16)  # [D, D] = [d, j]
    wkT = np.ascontiguousarray(w_k.T).astype(BF16)
    wvT = np.ascontiguousarray(w_v.T).astype(BF16)

    in_maps = []
    for c in range(N_CORES):
        b = c // 4
        hp = c % 4
        js = slice(hp * D2, (hp + 1) * D2)
        h0 = hp * D2
        in_maps.append(
            {
                "qT": qT[b],
                "kT": kTb[b],
                "vT": vTb[b],
                "wqT2": np.ascontiguousarray(wqT[:, js]),
                "wkT2": np.ascontiguousarray(wkT[:, js]),
                "wvT2": np.ascontiguousarray(wvT[:, js]),
                "wo0": np.ascontiguousarray(w_o[:, h0 : h0 + DH].T).astype(BF16),
                "wo1": np.ascontiguousarray(w_o[:, h0 + DH : h0 + 2 * DH].T).astype(BF16),
                "bq2": np.ascontiguousarray(b_q[js].reshape(D2, 1)),
                "bk2": np.ascontiguousarray(b_k[js].reshape(D2, 1)),
                "bv2": np.ascontiguousarray(b_v[js].reshape(1, D2)).astype(BF16),
            }
        )
    return in_maps


def gather_output(results, b_o):
    """Sum per-batch partials, add output bias, restore [B, S, D] layout."""
    b_o = np.asarray(b_o, np.float32)
    out = np.empty((B, S, D), np.float32)
    for b in range(B):
        acc = np.zeros((D, S), np.float32)
        for c in range(b * 4, b * 4 + 4):
            acc += results[c]["ouT"]
        out[b] = acc.T + b_o[None, :]
    return out


def kernel(q, k, v, w_q, b_q, w_k, b_k, w_v, b_v, w_o, b_o):
    nc = _get_nc()
    in_maps = make_in_maps(q, k, v, w_q, b_q, w_k, b_k, w_v, b_v, w_o, b_o)
    res = run_bass_kernel_spmd(nc, in_maps, core_ids=list(range(N_CORES)))
    return gather_output(res.results, b_o)


# revision 16
# speedup vs baseline: 1.1284x; 1.1284x over previous
"""Multi-head attention (B=2, S=4096, D=512, H=8) on 8 Trainium2 NeuronCores.

Sharding: batch x head-pair.  Core c handles batch b = c//4 and heads
(2*(c%4), 2*(c%4)+1).  Each core computes its heads' Q/K/V projections,
flash-style attention (scores kept transposed [kv, q] so the attn@V matmul
consumes the exp() output directly, with softmax denominators accumulated via
an extra ones-column on V), and its heads' slice of the output projection.
The 4 per-batch partial outputs are summed on the host (row-parallel linear)
and the output bias is added there.

All matmul operands are bf16 (f32 PSUM accumulation); inputs are transposed
and cast on the host so the device consumes [d, s]-layout activations
directly (no on-device transposes).
"""

import sys

sys.path.insert(0, "/opt/trn_rl_repo")

import numpy as np
import ml_dtypes

import concourse.bacc as bacc
import concourse.bass as bass
import concourse.tile as tile
from concourse import mybir
from concourse.bass_utils import run_bass_kernel_spmd

BF16 = ml_dtypes.bfloat16

B = 2
S = 4096
D = 512
H = 8
DH = 64           # head dim
HPC = 2           # heads per core
D2 = HPC * DH     # 128, the two heads' feature slice
N_CORES = 8
QC = 512          # query chunk (free dim of scores/attnV matmuls)
KVC = 128         # kv chunk (partition dim of transposed scores)
N_QC = S // QC    # 8
N_KVC = S // KVC  # 32
GROUP = 3         # kv chunks per exp() instruction (PSUM banks per S tile)

FP32 = mybir.dt.float32
BF16_T = mybir.dt.bfloat16
AF = mybir.ActivationFunctionType


def build_kernel():
    nc = bacc.Bacc("TRN2", debug=False, enable_asserts=False, num_devices=N_CORES)

    # DRAM I/O (per-core shapes; identical program on every core)
    qT = nc.dram_tensor("qT", [D, S], BF16_T, kind="ExternalInput").ap()
    kT = nc.dram_tensor("kT", [D, S], BF16_T, kind="ExternalInput").ap()
    vT = nc.dram_tensor("vT", [D, S], BF16_T, kind="ExternalInput").ap()
    wqT2 = nc.dram_tensor("wqT2", [D, D2], BF16_T, kind="ExternalInput").ap()
    wkT2 = nc.dram_tensor("wkT2", [D, D2], BF16_T, kind="ExternalInput").ap()
    wvT2 = nc.dram_tensor("wvT2", [D, D2], BF16_T, kind="ExternalInput").ap()
    wo0 = nc.dram_tensor("wo0", [DH, D], BF16_T, kind="ExternalInput").ap()
    wo1 = nc.dram_tensor("wo1", [DH, D], BF16_T, kind="ExternalInput").ap()
    bq2 = nc.dram_tensor("bq2", [D2, 1], FP32, kind="ExternalInput").ap()
    bk2 = nc.dram_tensor("bk2", [D2, 1], FP32, kind="ExternalInput").ap()
    bv2 = nc.dram_tensor("bv2", [1, D2], BF16_T, kind="ExternalInput").ap()
    ouT = nc.dram_tensor("ouT", [D, S], FP32, kind="ExternalOutput").ap()

    KD = D // 128  # 4 contraction chunks of 128

    with tile.TileContext(nc) as tc:
        with (
            tc.tile_pool(name="persist", bufs=1) as pp,
            tc.tile_pool(name="stream", bufs=6) as ps,
            tc.tile_pool(name="ptpool", bufs=3) as ppt,
            tc.tile_pool(name="norm", bufs=2) as pn,
            tc.tile_pool(name="outs", bufs=4) as po,
            tc.tile_pool(name="psum", bufs=1, space="PSUM") as psum,
        ):
            # ---- constants / weights to SBUF ----
            wq_sb = pp.tile([128, KD, D2], BF16_T)
            wk_sb = pp.tile([128, KD, D2], BF16_T)
            wv_sb = pp.tile([128, KD, D2], BF16_T)
            nc.sync.dma_start(out=wq_sb, in_=wqT2.rearrange("(c p) m -> p c m", p=128))
            nc.sync.dma_start(out=wk_sb, in_=wkT2.rearrange("(c p) m -> p c m", p=128))
            nc.sync.dma_start(out=wv_sb, in_=wvT2.rearrange("(c p) m -> p c m", p=128))
            wo_sb = [pp.tile([DH, D], BF16_T, tag=f"wo{h}", name=f"wo{h}") for h in range(HPC)]
            nc.sync.dma_start(out=wo_sb[0], in_=wo0)
            nc.sync.dma_start(out=wo_sb[1], in_=wo1)
            bq_sb = pp.tile([D2, 1], FP32, tag="bq")
            bk_sb = pp.tile([D2, 1], FP32, tag="bk")
            bv_sb = pp.tile([1, D2], BF16_T, tag="bv")
            nc.sync.dma_start(out=bq_sb, in_=bq2)
            nc.sync.dma_start(out=bk_sb, in_=bk2)
            nc.sync.dma_start(out=bv_sb, in_=bv2)
            ones_row = pp.tile([1, 128], BF16_T, tag="ones")
            nc.vector.memset(ones_row, 1.0)

            # ---- persistent activations ----
            qpT = pp.tile([D2, S], BF16_T, tag="qpT")   # [2*dh, s] both heads
            kpT = pp.tile([D2, S], BF16_T, tag="kpT")
            # vp per head: [kv in chunk, chunk, dh+1]; last col = ones (denom)
            vp = [pp.tile([128, N_KVC, 128], BF16_T, tag=f"vp{h}", name=f"vp{h}") for h in range(HPC)]
            for h in range(HPC):
                nc.vector.memset(vp[h][:, :, DH + 1 :], 0.0)
                nc.vector.memset(vp[h][:, :, DH : DH + 1], 1.0)

            # ---- stage A: projections ----
            # vp first: the flash loop's attnV needs it; q/k projections
            # overlap the first flash groups.
            # vp: out[s, j2] = x @ W.T  (lhsT = vT chunk (stationary), rhs = w chunk)
            vin = [ps.tile([128, S], BF16_T, tag="xin", name=f"vin_{kc}") for kc in range(KD)]
            for kc in range(KD):
                nc.sync.dma_start(out=vin[kc], in_=vT[kc * 128 : (kc + 1) * 128, :])
            for sc in range(N_KVC):
                pt = psum.tile([128, D2], FP32, tag=f"o{sc % 2}", name="pt_v")
                for kc in range(KD):
                    nc.tensor.matmul(
                        pt,
                        vin[kc][:, sc * 128 : (sc + 1) * 128],
                        wv_sb[:, kc, :],
                        start=(kc == 0),
                        stop=False,
                    )
                # bias via rank-1 update: ones[s] x bv[j2]
                nc.tensor.matmul(pt, ones_row, bv_sb, start=False, stop=True)
                for h in range(HPC):
                    nc.vector.tensor_copy(
                        out=vp[h][:, sc, 0:DH], in_=pt[:, h * DH : (h + 1) * DH]
                    )
            # qpT / kpT: out[j2, s] = W.T @ xT  (lhsT = w chunk, rhs = xT chunk)
            for name, src_, w_sb, b_sb, dst in (
                ("q", qT, wq_sb, bq_sb, qpT),
                ("k", kT, wk_sb, bk_sb, kpT),
            ):
                xin = [ps.tile([128, S], BF16_T, tag="xin", name=f"xin_{name}_{kc}") for kc in range(KD)]
                for kc in range(KD):
                    nc.sync.dma_start(out=xin[kc], in_=src_[kc * 128 : (kc + 1) * 128, :])
                for sc in range(S // 512):
                    pt = psum.tile([D2, 512], FP32, tag=f"s{sc % 2}", name="pt_qk")
                    for kc in range(KD):
                        nc.tensor.matmul(
                            pt,
                            w_sb[:, kc, :],
                            xin[kc][:, sc * 512 : (sc + 1) * 512],
                            start=(kc == 0),
                            stop=(kc == KD - 1),
                        )
                    # evict + per-partition bias on DVE (keeps ACT free for exp)
                    nc.vector.tensor_scalar_add(
                        out=dst[:, sc * 512 : (sc + 1) * 512],
                        in0=pt,
                        scalar1=b_sb,
                    )
            # ---- stage B: attention + output projection ----
            # symmetric kv-chunk groups of 3 PSUM banks per head
            # (3+3 score banks + 1+1 attn-out banks = 8; outproj reuses a
            # score slot by qc parity)
            groups = []
            kv = 0
            while kv < N_KVC:
                n = min(GROUP, N_KVC - kv)
                groups.append((kv, n))
                kv += n

            o_tiles = {}

            def make_attn(qc, g0, glen, p_sbs):
                def emit():
                    for gi in range(glen):
                        kvc = g0 + gi
                        for h in range(HPC):
                            nc.tensor.matmul(
                                o_tiles[(qc, h)],
                                vp[h][:, kvc, :],
                                p_sbs[h][:, gi * QC : (gi + 1) * QC],
                                start=(kvc == 0),
                                stop=(kvc == N_KVC - 1),
                            )
                return emit

            def make_norm(qc):
                def emit():
                    ous = []
                    den2 = pn.tile([1, HPC * QC], FP32, tag="den2", name="den2")
                    for h in range(HPC):
                        ou = pn.tile([DH, QC], FP32, tag=f"ou{h}", name=f"ou{h}")
                        nc.vector.tensor_copy(out=ou, in_=o_tiles[(qc, h)][0:DH, :])
                        nc.vector.tensor_copy(
                            out=den2[0:1, h * QC : (h + 1) * QC],
                            in_=o_tiles[(qc, h)][DH : DH + 1, :],
                        )
                        ous.append(ou)
                    rec2 = pn.tile([1, HPC * QC], FP32, tag="rec2", name="rec2")
                    nc.vector.reciprocal_approx_fast(out=rec2, in_=den2)
                    outn = []
                    for h in range(HPC):
                        bcast = pn.tile([DH, QC], FP32, tag=f"bcast{h}", name=f"bcast{h}")
                        nc.gpsimd.partition_broadcast(
                            bcast, rec2[0:1, h * QC : (h + 1) * QC]
                        )
                        on = pn.tile([DH, QC], BF16_T, tag=f"outn{h}", name=f"on{h}")
                        nc.vector.tensor_mul(on, ous[h], bcast)
                        outn.append(on)
                    return outn
                return emit

            def make_proj(qc, outn):
                def emit():
                    qs = slice(qc * QC, (qc + 1) * QC)
                    for ec in range(D // 128):
                        op = psum.tile([128, QC], FP32, tag=f"s{qc % 2}", name="op")
                        nc.tensor.matmul(
                            op, wo_sb[0][:, ec * 128 : (ec + 1) * 128], outn[0],
                            start=True, stop=False,
                        )
                        nc.tensor.matmul(
                            op, wo_sb[1][:, ec * 128 : (ec + 1) * 128], outn[1],
                            start=False, stop=True,
                        )
                        ot = po.tile([128, QC], FP32, tag="ot", name="ot")
                        nc.vector.tensor_copy(out=ot, in_=op)
                        nc.sync.dma_start(
                            out=ouT[ec * 128 : (ec + 1) * 128, qs], in_=ot
                        )
                return emit

            # software pipeline: attnV of each event is deferred one event so
            # the PE never sits behind the exp() it just fed; per-qc epilogue
            # (normalize + outproj) is deferred past that attnV.
            prev_attn = None
            pending_norm = None
            pending_proj = None
            for qc in range(N_QC):
                qs = slice(qc * QC, (qc + 1) * QC)
                for h in range(HPC):
                    o_tiles[(qc, h)] = psum.tile(
                        [128, QC], FP32, tag=f"o{h}", name=f"o_ps{h}"
                    )
                for evi, (g0, glen) in enumerate(groups):
                    s_tiles = [
                        psum.tile([128, glen * QC], FP32, tag=f"s{h}", name=f"s_ps{h}")
                        for h in range(HPC)
                    ]
                    for gi in range(glen):
                        kvc = g0 + gi
                        for h in range(HPC):
                            hs = slice(h * DH, (h + 1) * DH)
                            nc.tensor.matmul(
                                s_tiles[h][:, gi * QC : (gi + 1) * QC],
                                kpT[hs, kvc * KVC : (kvc + 1) * KVC],
                                qpT[hs, qs],
                                start=True,
                                stop=True,
                            )
                    p_sbs = []
                    for h in range(HPC):
                        p_sb = ppt.tile(
                            [128, glen * QC], BF16_T, tag=f"pt{h}", name=f"p_sb{h}"
                        )
                        nc.scalar.activation(
                            out=p_sb, in_=s_tiles[h], func=AF.Exp, scale=0.125
                        )
                        p_sbs.append(p_sb)
                    if prev_attn is not None:
                        prev_attn()
                    prev_attn = make_attn(qc, g0, glen, p_sbs)
                    if pending_norm is not None:
                        outn = pending_norm()
                        pending_proj = make_proj(qc - 1, outn)
                        pending_norm = None
                    elif pending_proj is not None and evi >= 6:
                        pending_proj()
                        pending_proj = None
                pending_norm = make_norm(qc)
            prev_attn()
            outn = pending_norm()
            make_proj(N_QC - 1, outn)()
    nc.compile()
    return nc


_NC_CACHE = None


def _get_nc():
    global _NC_CACHE
    if _NC_CACHE is None:
        _NC_CACHE = build_kernel()
    return _NC_CACHE


def make_in_maps(q, k, v, w_q, b_q, w_k, b_k, w_v, b_v, w_o, b_o):
    """Shard the full inputs into the 8 per-core input maps."""
    q = np.asarray(q, np.float32)
    k = np.asarray(k, np.float32)
    v = np.asarray(v, np.float32)
    w_q = np.asarray(w_q, np.float32)
    w_k = np.asarray(w_k, np.float32)
    w_v = np.asarray(w_v, np.float32)
    w_o = np.asarray(w_o, np.float32)
    b_q = np.asarray(b_q, np.float32)
    b_k = np.asarray(b_k, np.float32)
    b_v = np.asarray(b_v, np.float32)

    qT = [np.ascontiguousarray(q[b].T).astype(BF16) for b in range(B)]
    kTb = [np.ascontiguousarray(k[b].T).astype(BF16) for b in range(B)]
    vTb = [np.ascontiguousarray(v[b].T).astype(BF16) for b in range(B)]
    wqT = np.ascontiguousarray(w_q.T).astype(BF16)  # [D, D] = [d, j]
    wkT = np.ascontiguousarray(w_k.T).astype(BF16)
    wvT = np.ascontiguousarray(w_v.T).astype(BF16)

    in_maps = []
    for c in range(N_CORES):
        b = c // 4
        hp = c % 4
        js = slice(hp * D2, (hp + 1) * D2)
        h0 = hp * D2
        in_maps.append(
            {
                "qT": qT[b],
                "kT": kTb[b],
                "vT": vTb[b],
                "wqT2": np.ascontiguousarray(wqT[:, js]),
                "wkT2": np.ascontiguousarray(wkT[:, js]),
                "wvT2": np.ascontiguousarray(wvT[:, js]),
                "wo0": np.ascontiguousarray(w_o[:, h0 : h0 + DH].T).astype(BF16),
                "wo1": np.ascontiguousarray(w_o[:, h0 + DH : h0 + 2 * DH].T).astype(BF16),
                "bq2": np.ascontiguousarray(b_q[js].reshape(D2, 1)),
                "bk2": np.ascontiguousarray(b_k[js].reshape(D2, 1)),
                "bv2": np.ascontiguousarray(b_v[js].reshape(1, D2)).astype(BF16),
            }
        )
    return in_maps


def gather_output(results, b_o):
    """Sum per-batch partials, add output bias, restore [B, S, D] layout."""
    b_o = np.asarray(b_o, np.float32)
    out = np.empty((B, S, D), np.float32)
    for b in range(B):
        acc = np.zeros((D, S), np.float32)
        for c in range(b * 4, b * 4 + 4):
            acc += results[c]["ouT"]
        out[b] = acc.T + b_o[None, :]
    return out


def kernel(q, k, v, w_q, b_q, w_k, b_k, w_v, b_v, w_o, b_o):
    nc = _get_nc()
    in_maps = make_in_maps(q, k, v, w_q, b_q, w_k, b_k, w_v, b_v, w_o, b_o)
    res = run_bass_kernel_spmd(nc, in_maps, core_ids=list(range(N_CORES)))
    return gather_output(res.results, b_o)


# revision 17
# speedup vs baseline: 1.1422x; 1.0123x over previous
"""Multi-head attention (B=2, S=4096, D=512, H=8) on 8 Trainium2 NeuronCores.

Sharding: batch x head-pair.  Core c handles batch b = c//4 and heads
(2*(c%4), 2*(c%4)+1).  Each core computes its heads' Q/K/V projections,
flash-style attention (scores kept transposed [kv, q] so the attn@V matmul
consumes the exp() output directly, with softmax denominators accumulated via
an extra ones-column on V), and its heads' slice of the output projection.
The 4 per-batch partial outputs are summed on the host (row-parallel linear)
and the output bias is added there.

All matmul operands are bf16 (f32 PSUM accumulation); inputs are transposed
and cast on the host so the device consumes [d, s]-layout activations
directly (no on-device transposes).
"""

import sys

sys.path.insert(0, "/opt/trn_rl_repo")

import numpy as np
import ml_dtypes

import concourse.bacc as bacc
import concourse.bass as bass
import concourse.tile as tile
from concourse import mybir
from concourse.bass_utils import run_bass_kernel_spmd

BF16 = ml_dtypes.bfloat16

B = 2
S = 4096
D = 512
H = 8
DH = 64           # head dim
HPC = 2           # heads per core
D2 = HPC * DH     # 128, the two heads' feature slice
N_CORES = 8
QC = 512          # query chunk (free dim of scores/attnV matmuls)
KVC = 128         # kv chunk (partition dim of transposed scores)
N_QC = S // QC    # 8
N_KVC = S // KVC  # 32
GROUP = 3         # kv chunks per exp() instruction (PSUM banks per S tile)

FP32 = mybir.dt.float32
BF16_T = mybir.dt.bfloat16
AF = mybir.ActivationFunctionType


def build_kernel():
    nc = bacc.Bacc("TRN2", debug=False, enable_asserts=False, num_devices=N_CORES)

    # DRAM I/O (per-core shapes; identical program on every core)
    qT = nc.dram_tensor("qT", [D, S], BF16_T, kind="ExternalInput").ap()
    kT = nc.dram_tensor("kT", [D, S], BF16_T, kind="ExternalInput").ap()
    vT = nc.dram_tensor("vT", [D, S], BF16_T, kind="ExternalInput").ap()
    wqT2 = nc.dram_tensor("wqT2", [D, D2], BF16_T, kind="ExternalInput").ap()
    wkT2 = nc.dram_tensor("wkT2", [D, D2], BF16_T, kind="ExternalInput").ap()
    wvT2 = nc.dram_tensor("wvT2", [D, D2], BF16_T, kind="ExternalInput").ap()
    wo0 = nc.dram_tensor("wo0", [DH, D], BF16_T, kind="ExternalInput").ap()
    wo1 = nc.dram_tensor("wo1", [DH, D], BF16_T, kind="ExternalInput").ap()
    bq2 = nc.dram_tensor("bq2", [D2, 1], FP32, kind="ExternalInput").ap()
    bk2 = nc.dram_tensor("bk2", [D2, 1], FP32, kind="ExternalInput").ap()
    bv2 = nc.dram_tensor("bv2", [1, D2], BF16_T, kind="ExternalInput").ap()
    ouT = nc.dram_tensor("ouT", [D, S], FP32, kind="ExternalOutput").ap()

    KD = D // 128  # 4 contraction chunks of 128

    with tile.TileContext(nc) as tc:
        with (
            tc.tile_pool(name="persist", bufs=1) as pp,
            tc.tile_pool(name="stream", bufs=6) as ps,
            tc.tile_pool(name="ptpool", bufs=3) as ppt,
            tc.tile_pool(name="norm", bufs=2) as pn,
            tc.tile_pool(name="outs", bufs=4) as po,
            tc.tile_pool(name="psum", bufs=1, space="PSUM") as psum,
        ):
            # ---- constants / weights to SBUF ----
            wq_sb = pp.tile([128, KD, D2], BF16_T)
            wk_sb = pp.tile([128, KD, D2], BF16_T)
            wv_sb = pp.tile([128, KD, D2], BF16_T)
            nc.sync.dma_start(out=wq_sb, in_=wqT2.rearrange("(c p) m -> p c m", p=128))
            nc.sync.dma_start(out=wk_sb, in_=wkT2.rearrange("(c p) m -> p c m", p=128))
            nc.sync.dma_start(out=wv_sb, in_=wvT2.rearrange("(c p) m -> p c m", p=128))
            wo_sb = [pp.tile([DH, D], BF16_T, tag=f"wo{h}", name=f"wo{h}") for h in range(HPC)]
            nc.sync.dma_start(out=wo_sb[0], in_=wo0)
            nc.sync.dma_start(out=wo_sb[1], in_=wo1)
            bq_sb = pp.tile([D2, 1], FP32, tag="bq")
            bk_sb = pp.tile([D2, 1], FP32, tag="bk")
            bv_sb = pp.tile([1, D2], BF16_T, tag="bv")
            nc.sync.dma_start(out=bq_sb, in_=bq2)
            nc.sync.dma_start(out=bk_sb, in_=bk2)
            nc.sync.dma_start(out=bv_sb, in_=bv2)
            ones_row = pp.tile([1, 128], BF16_T, tag="ones")
            nc.vector.memset(ones_row, 1.0)

            # ---- persistent activations ----
            qpT = pp.tile([D2, S], BF16_T, tag="qpT")   # [2*dh, s] both heads
            kpT = pp.tile([D2, S], BF16_T, tag="kpT")
            # vp per head: [kv in chunk, chunk, dh+1]; last col = ones (denom)
            vp = [pp.tile([128, N_KVC, 128], BF16_T, tag=f"vp{h}", name=f"vp{h}") for h in range(HPC)]
            for h in range(HPC):
                nc.vector.memset(vp[h][:, :, DH + 1 :], 0.0)
                nc.vector.memset(vp[h][:, :, DH : DH + 1], 1.0)

            # ---- stage A: projections ----
            # qpT / kpT: out[j2, s] = W.T @ xT  (lhsT = w chunk, rhs = xT chunk)
            for name, src_, w_sb, b_sb, dst in (
                ("q", qT, wq_sb, bq_sb, qpT),
                ("k", kT, wk_sb, bk_sb, kpT),
            ):
                xin = [ps.tile([128, S], BF16_T, tag="xin", name=f"xin_{name}_{kc}") for kc in range(KD)]
                for kc in range(KD):
                    nc.sync.dma_start(out=xin[kc], in_=src_[kc * 128 : (kc + 1) * 128, :])
                for sc in range(S // 512):
                    pt = psum.tile([D2, 512], FP32, tag=f"s{sc % 2}", name="pt_qk")
                    for kc in range(KD):
                        nc.tensor.matmul(
                            pt,
                            w_sb[:, kc, :],
                            xin[kc][:, sc * 512 : (sc + 1) * 512],
                            start=(kc == 0),
                            stop=(kc == KD - 1),
                        )
                    # evict + per-partition bias on DVE (keeps ACT free for exp)
                    nc.vector.tensor_scalar_add(
                        out=dst[:, sc * 512 : (sc + 1) * 512],
                        in0=pt,
                        scalar1=b_sb,
                    )
            # vp: out[s, j2] = x @ W.T  (lhsT = vT chunk (stationary), rhs = w chunk)
            vin = [ps.tile([128, S], BF16_T, tag="xin", name=f"vin_{kc}") for kc in range(KD)]
            for kc in range(KD):
                nc.sync.dma_start(out=vin[kc], in_=vT[kc * 128 : (kc + 1) * 128, :])
            for sc in range(N_KVC):
                pt = psum.tile([128, D2], FP32, tag=f"o{sc % 2}", name="pt_v")
                for kc in range(KD):
                    nc.tensor.matmul(
                        pt,
                        vin[kc][:, sc * 128 : (sc + 1) * 128],
                        wv_sb[:, kc, :],
                        start=(kc == 0),
                        stop=False,
                    )
                # bias via rank-1 update: ones[s] x bv[j2]
                nc.tensor.matmul(pt, ones_row, bv_sb, start=False, stop=True)
                for h in range(HPC):
                    nc.vector.tensor_copy(
                        out=vp[h][:, sc, 0:DH], in_=pt[:, h * DH : (h + 1) * DH]
                    )
            # ---- stage B: attention + output projection ----
            # symmetric kv-chunk groups of 3 PSUM banks per head
            # (3+3 score banks + 1+1 attn-out banks = 8; outproj reuses a
            # score slot by qc parity)
            groups = []
            kv = 0
            while kv < N_KVC:
                n = min(GROUP, N_KVC - kv)
                groups.append((kv, n))
                kv += n

            o_tiles = {}

            def make_attn(qc, g0, glen, p_sbs):
                def emit():
                    for gi in range(glen):
                        kvc = g0 + gi
                        for h in range(HPC):
                            nc.tensor.matmul(
                                o_tiles[(qc, h)],
                                vp[h][:, kvc, :],
                                p_sbs[h][:, gi * QC : (gi + 1) * QC],
                                start=(kvc == 0),
                                stop=(kvc == N_KVC - 1),
                            )
                return emit

            def make_norm(qc):
                def emit():
                    ous = []
                    den2 = pn.tile([1, HPC * QC], FP32, tag="den2", name="den2")
                    for h in range(HPC):
                        ou = pn.tile([DH, QC], FP32, tag=f"ou{h}", name=f"ou{h}")
                        nc.vector.tensor_copy(out=ou, in_=o_tiles[(qc, h)][0:DH, :])
                        nc.vector.tensor_copy(
                            out=den2[0:1, h * QC : (h + 1) * QC],
                            in_=o_tiles[(qc, h)][DH : DH + 1, :],
                        )
                        ous.append(ou)
                    rec2 = pn.tile([1, HPC * QC], FP32, tag="rec2", name="rec2")
                    nc.vector.reciprocal_approx_fast(out=rec2, in_=den2)
                    outn = []
                    for h in range(HPC):
                        bcast = pn.tile([DH, QC], FP32, tag=f"bcast{h}", name=f"bcast{h}")
                        nc.gpsimd.partition_broadcast(
                            bcast, rec2[0:1, h * QC : (h + 1) * QC]
                        )
                        on = pn.tile([DH, QC], BF16_T, tag=f"outn{h}", name=f"on{h}")
                        nc.vector.tensor_mul(on, ous[h], bcast)
                        outn.append(on)
                    return outn
                return emit

            def make_proj(qc, outn):
                def emit():
                    qs = slice(qc * QC, (qc + 1) * QC)
                    for ec in range(D // 128):
                        op = psum.tile([128, QC], FP32, tag=f"s{qc % 2}", name="op")
                        nc.tensor.matmul(
                            op, wo_sb[0][:, ec * 128 : (ec + 1) * 128], outn[0],
                            start=True, stop=False,
                        )
                        nc.tensor.matmul(
                            op, wo_sb[1][:, ec * 128 : (ec + 1) * 128], outn[1],
                            start=False, stop=True,
                        )
                        ot = po.tile([128, QC], FP32, tag="ot", name="ot")
                        nc.vector.tensor_copy(out=ot, in_=op)
                        nc.sync.dma_start(
                            out=ouT[ec * 128 : (ec + 1) * 128, qs], in_=ot
                        )
                return emit

            # software pipeline: attnV of each event is deferred one event so
            # the PE never sits behind the exp() it just fed; per-qc epilogue
            # (normalize + outproj) is deferred past that attnV.
            prev_attn = None
            pending_norm = None
            pending_proj = None
            for qc in range(N_QC):
                qs = slice(qc * QC, (qc + 1) * QC)
                for h in range(HPC):
                    o_tiles[(qc, h)] = psum.tile(
                        [128, QC], FP32, tag=f"o{h}", name=f"o_ps{h}"
                    )
                for evi, (g0, glen) in enumerate(groups):
                    s_tiles = [
                        psum.tile([128, glen * QC], FP32, tag=f"s{h}", name=f"s_ps{h}")
                        for h in range(HPC)
                    ]
                    for gi in range(glen):
                        kvc = g0 + gi
                        for h in range(HPC):
                            hs = slice(h * DH, (h + 1) * DH)
                            nc.tensor.matmul(
                                s_tiles[h][:, gi * QC : (gi + 1) * QC],
                                kpT[hs, kvc * KVC : (kvc + 1) * KVC],
                                qpT[hs, qs],
                                start=True,
                                stop=True,
                            )
                    p_sbs = []
                    for h in range(HPC):
                        p_sb = ppt.tile(
                            [128, glen * QC], BF16_T, tag=f"pt{h}", name=f"p_sb{h}"
                        )
                        nc.scalar.activation(
                            out=p_sb, in_=s_tiles[h], func=AF.Exp, scale=0.125
                        )
                        p_sbs.append(p_sb)
                    if prev_attn is not None:
                        prev_attn()
                    prev_attn = make_attn(qc, g0, glen, p_sbs)
                    if pending_norm is not None:
                        outn = pending_norm()
                        pending_proj = make_proj(qc - 1, outn)
                        pending_norm = None
                    elif pending_proj is not None and evi >= 6:
                        pending_proj()
                        pending_proj = None
                pending_norm = make_norm(qc)
            prev_attn()
            outn = pending_norm()
            make_proj(N_QC - 1, outn)()
    nc.compile()
    return nc


_NC_CACHE = None


def _get_nc():
    global _NC_CACHE
    if _NC_CACHE is None:
        _NC_CACHE = build_kernel()
    return _NC_CACHE


def make_in_maps(q, k, v, w_q, b_q, w_k, b_k, w_v, b_v, w_o, b_o):
    """Shard the full inputs into the 8 per-core input maps."""
    q = np.asarray(q, np.float32)
    k = np.asarray(k, np.float32)
    v = np.asarray(v, np.float32)
    w_q = np.asarray(w_q, np.float32)
    w_k = np.asarray(w_k, np.float32)
    w_v = np.asarray(w_v, np.float32)
    w_o = np.asarray(w_o, np.float32)
    b_q = np.asarray(b_q, np.float32)
    b_k = np.asarray(b_k, np.float32)
    b_v = np.asarray(b_v, np.float32)

    qT = [np.ascontiguousarray(q[b].T).astype(BF16) for b in range(B)]
    kTb = [np.ascontiguousarray(k[b].T).astype(BF16) for b in range(B)]
    vTb = [np.ascontiguousarray(v[b].T).astype(BF16) for b in range(B)]
    wqT = np.ascontiguousarray(w_q.T).astype(BF16)  # [D, D] = [d, j]
    wkT = np.ascontiguousarray(w_k.T).astype(BF16)
    wvT = np.ascontiguousarray(w_v.T).astype(BF16)

    in_maps = []
    for c in range(N_CORES):
        b = c // 4
        hp = c % 4
        js = slice(hp * D2, (hp + 1) * D2)
        h0 = hp * D2
        in_maps.append(
            {
                "qT": qT[b],
                "kT": kTb[b],
                "vT": vTb[b],
                "wqT2": np.ascontiguousarray(wqT[:, js]),
                "wkT2": np.ascontiguousarray(wkT[:, js]),
                "wvT2": np.ascontiguousarray(wvT[:, js]),
                "wo0": np.ascontiguousarray(w_o[:, h0 : h0 + DH].T).astype(BF16),
                "wo1": np.ascontiguousarray(w_o[:, h0 + DH : h0 + 2 * DH].T).astype(BF16),
                "bq2": np.ascontiguousarray(b_q[js].reshape(D2, 1)),
                "bk2": np.ascontiguousarray(b_k[js].reshape(D2, 1)),
                "bv2": np.ascontiguousarray(b_v[js].reshape(1, D2)).astype(BF16),
            }
        )
    return in_maps


def gather_output(results, b_o):
    """Sum per-batch partials, add output bias, restore [B, S, D] layout."""
    b_o = np.asarray(b_o, np.float32)
    out = np.empty((B, S, D), np.float32)
    for b in range(B):
        acc = np.zeros((D, S), np.float32)
        for c in range(b * 4, b * 4 + 4):
            acc += results[c]["ouT"]
        out[b] = acc.T + b_o[None, :]
    return out


def kernel(q, k, v, w_q, b_q, w_k, b_k, w_v, b_v, w_o, b_o):
    nc = _get_nc()
    in_maps = make_in_maps(q, k, v, w_q, b_q, w_k, b_k, w_v, b_v, w_o, b_o)
    res = run_bass_kernel_spmd(nc, in_maps, core_ids=list(range(N_CORES)))
    return gather_output(res.results, b_o)


# revision 18
# speedup vs baseline: 1.1523x; 1.0088x over previous
"""Multi-head attention (B=2, S=4096, D=512, H=8) on 8 Trainium2 NeuronCores.

Sharding: batch x head-pair.  Core c handles batch b = c//4 and heads
(2*(c%4), 2*(c%4)+1).  Each core computes its heads' Q/K/V projections,
flash-style attention (scores kept transposed [kv, q] so the attn@V matmul
consumes the exp() output directly, with softmax denominators accumulated via
an extra ones-column on V), and its heads' slice of the output projection.
The 4 per-batch partial outputs are summed on the host (row-parallel linear)
and the output bias is added there.

All matmul operands are bf16 (f32 PSUM accumulation); inputs are transposed
and cast on the host so the device consumes [d, s]-layout activations
directly (no on-device transposes).
"""

import sys

sys.path.insert(0, "/opt/trn_rl_repo")

import numpy as np
import ml_dtypes

import concourse.bacc as bacc
import concourse.bass as bass
import concourse.tile as tile
from concourse import mybir
from concourse.bass_utils import run_bass_kernel_spmd

BF16 = ml_dtypes.bfloat16

B = 2
S = 4096
D = 512
H = 8
DH = 64           # head dim
HPC = 2           # heads per core
D2 = HPC * DH     # 128, the two heads' feature slice
N_CORES = 8
QC = 512          # query chunk (free dim of scores/attnV matmuls)
KVC = 128         # kv chunk (partition dim of transposed scores)
N_QC = S // QC    # 8
N_KVC = S // KVC  # 32
GROUP = 3         # kv chunks per exp() instruction (PSUM banks per S tile)

FP32 = mybir.dt.float32
BF16_T = mybir.dt.bfloat16
AF = mybir.ActivationFunctionType


def build_kernel():
    nc = bacc.Bacc("TRN2", debug=False, enable_asserts=False, num_devices=N_CORES)

    # DRAM I/O (per-core shapes; identical program on every core)
    qT = nc.dram_tensor("qT", [D, S], BF16_T, kind="ExternalInput").ap()
    kT = nc.dram_tensor("kT", [D, S], BF16_T, kind="ExternalInput").ap()
    vT = nc.dram_tensor("vT", [D, S], BF16_T, kind="ExternalInput").ap()
    wqT2 = nc.dram_tensor("wqT2", [D, D2], BF16_T, kind="ExternalInput").ap()
    wkT2 = nc.dram_tensor("wkT2", [D, D2], BF16_T, kind="ExternalInput").ap()
    wvT2 = nc.dram_tensor("wvT2", [D, D2], BF16_T, kind="ExternalInput").ap()
    wo0 = nc.dram_tensor("wo0", [DH, D], BF16_T, kind="ExternalInput").ap()
    wo1 = nc.dram_tensor("wo1", [DH, D], BF16_T, kind="ExternalInput").ap()
    bq2 = nc.dram_tensor("bq2", [D2, 1], FP32, kind="ExternalInput").ap()
    bk2 = nc.dram_tensor("bk2", [D2, 1], FP32, kind="ExternalInput").ap()
    bv2 = nc.dram_tensor("bv2", [1, D2], BF16_T, kind="ExternalInput").ap()
    ouT = nc.dram_tensor("ouT", [D, S], FP32, kind="ExternalOutput").ap()

    KD = D // 128  # 4 contraction chunks of 128

    with tile.TileContext(nc) as tc:
        with (
            tc.tile_pool(name="persist", bufs=1) as pp,
            tc.tile_pool(name="stream", bufs=6) as ps,
            tc.tile_pool(name="ptpool", bufs=3) as ppt,
            tc.tile_pool(name="norm", bufs=2) as pn,
            tc.tile_pool(name="outs", bufs=4) as po,
            tc.tile_pool(name="psum", bufs=1, space="PSUM") as psum,
        ):
            # ---- constants / weights to SBUF ----
            wq_sb = pp.tile([128, KD, D2], BF16_T)
            wk_sb = pp.tile([128, KD, D2], BF16_T)
            wv_sb = pp.tile([128, KD, D2], BF16_T)
            nc.sync.dma_start(out=wq_sb, in_=wqT2.rearrange("(c p) m -> p c m", p=128))
            nc.sync.dma_start(out=wk_sb, in_=wkT2.rearrange("(c p) m -> p c m", p=128))
            nc.sync.dma_start(out=wv_sb, in_=wvT2.rearrange("(c p) m -> p c m", p=128))
            wo_sb = [pp.tile([DH, D], BF16_T, tag=f"wo{h}", name=f"wo{h}") for h in range(HPC)]
            nc.sync.dma_start(out=wo_sb[0], in_=wo0)
            nc.sync.dma_start(out=wo_sb[1], in_=wo1)
            bq_sb = pp.tile([D2, 1], FP32, tag="bq")
            bk_sb = pp.tile([D2, 1], FP32, tag="bk")
            bv_sb = pp.tile([1, D2], BF16_T, tag="bv")
            nc.sync.dma_start(out=bq_sb, in_=bq2)
            nc.sync.dma_start(out=bk_sb, in_=bk2)
            nc.sync.dma_start(out=bv_sb, in_=bv2)
            ones_row = pp.tile([1, 128], BF16_T, tag="ones")
            nc.vector.memset(ones_row, 1.0)

            # ---- persistent activations ----
            qpT = pp.tile([D2, S], BF16_T, tag="qpT")   # [2*dh, s] both heads
            kpT = pp.tile([D2, S], BF16_T, tag="kpT")
            # vp per head: [kv in chunk, chunk, dh+1]; last col = ones (denom)
            vp = [pp.tile([128, N_KVC, 128], BF16_T, tag=f"vp{h}", name=f"vp{h}") for h in range(HPC)]
            for h in range(HPC):
                nc.vector.memset(vp[h][:, :, DH + 1 :], 0.0)
                nc.vector.memset(vp[h][:, :, DH : DH + 1], 1.0)

            # ---- stage A: projections ----
            # qpT / kpT: out[j2, s] = W.T @ xT  (lhsT = w chunk, rhs = xT chunk)
            for name, src_, w_sb, b_sb, dst in (
                ("q", qT, wq_sb, bq_sb, qpT),
                ("k", kT, wk_sb, bk_sb, kpT),
            ):
                xin = [ps.tile([128, S], BF16_T, tag="xin", name=f"xin_{name}_{kc}") for kc in range(KD)]
                for sb in range(4):
                    ss = slice(sb * (S // 4), (sb + 1) * (S // 4))
                    for kc in range(KD):
                        nc.sync.dma_start(
                            out=xin[kc][:, ss], in_=src_[kc * 128 : (kc + 1) * 128, ss]
                        )
                for sc in range(S // 512):
                    pt = psum.tile([D2, 512], FP32, tag=f"s{sc % 2}", name="pt_qk")
                    for kc in range(KD):
                        nc.tensor.matmul(
                            pt,
                            w_sb[:, kc, :],
                            xin[kc][:, sc * 512 : (sc + 1) * 512],
                            start=(kc == 0),
                            stop=(kc == KD - 1),
                        )
                    # evict + per-partition bias on DVE (keeps ACT free for exp)
                    nc.vector.tensor_scalar_add(
                        out=dst[:, sc * 512 : (sc + 1) * 512],
                        in0=pt,
                        scalar1=b_sb,
                    )
            # vp: out[s, j2] = x @ W.T  (lhsT = vT chunk (stationary), rhs = w chunk)
            vin = [ps.tile([128, S], BF16_T, tag="xin", name=f"vin_{kc}") for kc in range(KD)]
            for sb in range(4):
                ss = slice(sb * (S // 4), (sb + 1) * (S // 4))
                for kc in range(KD):
                    nc.sync.dma_start(
                        out=vin[kc][:, ss], in_=vT[kc * 128 : (kc + 1) * 128, ss]
                    )
            for sc in range(N_KVC):
                pt = psum.tile([128, D2], FP32, tag=f"o{sc % 2}", name="pt_v")
                for kc in range(KD):
                    nc.tensor.matmul(
                        pt,
                        vin[kc][:, sc * 128 : (sc + 1) * 128],
                        wv_sb[:, kc, :],
                        start=(kc == 0),
                        stop=False,
                    )
                # bias via rank-1 update: ones[s] x bv[j2]
                nc.tensor.matmul(pt, ones_row, bv_sb, start=False, stop=True)
                for h in range(HPC):
                    nc.vector.tensor_copy(
                        out=vp[h][:, sc, 0:DH], in_=pt[:, h * DH : (h + 1) * DH]
                    )
            # ---- stage B: attention + output projection ----
            # symmetric kv-chunk groups of 3 PSUM banks per head
            # (3+3 score banks + 1+1 attn-out banks = 8; outproj reuses a
            # score slot by qc parity)
            groups = []
            kv = 0
            while kv < N_KVC:
                n = min(GROUP, N_KVC - kv)
                groups.append((kv, n))
                kv += n

            o_tiles = {}

            def make_attn(qc, g0, glen, p_sbs):
                def emit():
                    for gi in range(glen):
                        kvc = g0 + gi
                        for h in range(HPC):
                            nc.tensor.matmul(
                                o_tiles[(qc, h)],
                                vp[h][:, kvc, :],
                                p_sbs[h][:, gi * QC : (gi + 1) * QC],
                                start=(kvc == 0),
                                stop=(kvc == N_KVC - 1),
                            )
                return emit

            def make_norm(qc):
                def emit():
                    ous = []
                    den2 = pn.tile([1, HPC * QC], FP32, tag="den2", name="den2")
                    for h in range(HPC):
                        ou = pn.tile([DH, QC], FP32, tag=f"ou{h}", name=f"ou{h}")
                        nc.vector.tensor_copy(out=ou, in_=o_tiles[(qc, h)][0:DH, :])
                        nc.vector.tensor_copy(
                            out=den2[0:1, h * QC : (h + 1) * QC],
                            in_=o_tiles[(qc, h)][DH : DH + 1, :],
                        )
                        ous.append(ou)
                    rec2 = pn.tile([1, HPC * QC], FP32, tag="rec2", name="rec2")
                    nc.vector.reciprocal_approx_fast(out=rec2, in_=den2)
                    outn = []
                    for h in range(HPC):
                        bcast = pn.tile([DH, QC], FP32, tag=f"bcast{h}", name=f"bcast{h}")
                        nc.gpsimd.partition_broadcast(
                            bcast, rec2[0:1, h * QC : (h + 1) * QC]
                        )
                        on = pn.tile([DH, QC], BF16_T, tag=f"outn{h}", name=f"on{h}")
                        nc.vector.tensor_mul(on, ous[h], bcast)
                        outn.append(on)
                    return outn
                return emit

            def make_proj(qc, outn):
                def emit():
                    qs = slice(qc * QC, (qc + 1) * QC)
                    for ec in range(D // 128):
                        op = psum.tile([128, QC], FP32, tag=f"s{qc % 2}", name="op")
                        nc.tensor.matmul(
                            op, wo_sb[0][:, ec * 128 : (ec + 1) * 128], outn[0],
                            start=True, stop=False,
                        )
                        nc.tensor.matmul(
                            op, wo_sb[1][:, ec * 128 : (ec + 1) * 128], outn[1],
                            start=False, stop=True,
                        )
                        ot = po.tile([128, QC], FP32, tag="ot", name="ot")
                        nc.vector.tensor_copy(out=ot, in_=op)
                        nc.sync.dma_start(
                            out=ouT[ec * 128 : (ec + 1) * 128, qs], in_=ot
                        )
                return emit

            # software pipeline: attnV of each event is deferred one event so
            # the PE never sits behind the exp() it just fed; per-qc epilogue
            # (normalize + outproj) is deferred past that attnV.
            prev_attn = None
            pending_norm = None
            pending_proj = None
            for qc in range(N_QC):
                qs = slice(qc * QC, (qc + 1) * QC)
                for h in range(HPC):
                    o_tiles[(qc, h)] = psum.tile(
                        [128, QC], FP32, tag=f"o{h}", name=f"o_ps{h}"
                    )
                for evi, (g0, glen) in enumerate(groups):
                    s_tiles = [
                        psum.tile([128, glen * QC], FP32, tag=f"s{h}", name=f"s_ps{h}")
                        for h in range(HPC)
                    ]
                    for gi in range(glen):
                        kvc = g0 + gi
                        for h in range(HPC):
                            hs = slice(h * DH, (h + 1) * DH)
                            nc.tensor.matmul(
                                s_tiles[h][:, gi * QC : (gi + 1) * QC],
                                kpT[hs, kvc * KVC : (kvc + 1) * KVC],
                                qpT[hs, qs],
                                start=True,
                                stop=True,
                            )
                    p_sbs = []
                    for h in range(HPC):
                        p_sb = ppt.tile(
                            [128, glen * QC], BF16_T, tag=f"pt{h}", name=f"p_sb{h}"
                        )
                        nc.scalar.activation(
                            out=p_sb, in_=s_tiles[h], func=AF.Exp, scale=0.125
                        )
                        p_sbs.append(p_sb)
                    if prev_attn is not None:
                        prev_attn()
                    prev_attn = make_attn(qc, g0, glen, p_sbs)
                    if pending_norm is not None:
                        outn = pending_norm()
                        pending_proj = make_proj(qc - 1, outn)
                        pending_norm = None
                    elif pending_proj is not None and evi >= 6:
                        pending_proj()
                        pending_proj = None
                pending_norm = make_norm(qc)
            prev_attn()
            outn = pending_norm()
            make_proj(N_QC - 1, outn)()
    nc.compile()
    return nc


_NC_CACHE = None


def _get_nc():
    global _NC_CACHE
    if _NC_CACHE is None:
        _NC_CACHE = build_kernel()
    return _NC_CACHE


def make_in_maps(q, k, v, w_q, b_q, w_k, b_k, w_v, b_v, w_o, b_o):
    """Shard the full inputs into the 8 per-core input maps."""
    q = np.asarray(q, np.float32)
    k = np.asarray(k, np.float32)
    v = np.asarray(v, np.float32)
    w_q = np.asarray(w_q, np.float32)
    w_k = np.asarray(w_k, np.float32)
    w_v = np.asarray(w_v, np.float32)
    w_o = np.asarray(w_o, np.float32)
    b_q = np.asarray(b_q, np.float32)
    b_k = np.asarray(b_k, np.float32)
    b_v = np.asarray(b_v, np.float32)

    qT = [np.ascontiguousarray(q[b].T).astype(BF16) for b in range(B)]
    kTb = [np.ascontiguousarray(k[b].T).astype(BF16) for b in range(B)]
    vTb = [np.ascontiguousarray(v[b].T).astype(BF16) for b in range(B)]
    wqT = np.ascontiguousarray(w_q.T).astype(BF16)  # [D, D] = [d, j]
    wkT = np.ascontiguousarray(w_k.T).astype(BF16)
    wvT = np.ascontiguousarray(w_v.T).astype(BF16)

    in_maps = []
    for c in range(N_CORES):
        b = c // 4
        hp = c % 4
        js = slice(hp * D2, (hp + 1) * D2)
        h0 = hp * D2
        in_maps.append(
            {
                "qT": qT[b],
                "kT": kTb[b],
                "vT": vTb[b],
                "wqT2": np.ascontiguousarray(wqT[:, js]),
                "wkT2": np.ascontiguousarray(wkT[:, js]),
                "wvT2": np.ascontiguousarray(wvT[:, js]),
                "wo0": np.ascontiguousarray(w_o[:, h0 : h0 + DH].T).astype(BF16),
                "wo1": np.ascontiguousarray(w_o[:, h0 + DH : h0 + 2 * DH].T).astype(BF16),
                "bq2": np.ascontiguousarray(b_q[js].reshape(D2, 1)),
                "bk2": np.ascontiguousarray(b_k[js].reshape(D2, 1)),
                "bv2": np.ascontiguousarray(b_v[js].reshape(1, D2)).astype(BF16),
            }
        )
    return in_maps


def gather_output(results, b_o):
    """Sum per-batch partials, add output bias, restore [B, S, D] layout."""
    b_o = np.asarray(b_o, np.float32)
    out = np.empty((B, S, D), np.float32)
    for b in range(B):
        acc = np.zeros((D, S), np.float32)
        for c in range(b * 4, b * 4 + 4):
            acc += results[c]["ouT"]
        out[b] = acc.T + b_o[None, :]
    return out


def kernel(q, k, v, w_q, b_q, w_k, b_k, w_v, b_v, w_o, b_o):
    nc = _get_nc()
    in_maps = make_in_maps(q, k, v, w_q, b_q, w_k, b_k, w_v, b_v, w_o, b_o)
    res = run_bass_kernel_spmd(nc, in_maps, core_ids=list(range(N_CORES)))
    return gather_output(res.results, b_o)


# revision 19
# speedup vs baseline: 1.1644x; 1.0105x over previous
"""Multi-head attention (B=2, S=4096, D=512, H=8) on 8 Trainium2 NeuronCores.

Sharding: batch x head-pair.  Core c handles batch b = c//4 and heads
(2*(c%4), 2*(c%4)+1).  Each core computes its heads' Q/K/V projections,
flash-style attention (scores kept transposed [kv, q] so the attn@V matmul
consumes the exp() output directly, with softmax denominators accumulated via
an extra ones-column on V), and its heads' slice of the output projection.
The 4 per-batch partial outputs are summed on the host (row-parallel linear)
and the output bias is added there.

All matmul operands are bf16 (f32 PSUM accumulation); inputs are transposed
and cast on the host so the device consumes [d, s]-layout activations
directly (no on-device transposes).
"""

import sys

sys.path.insert(0, "/opt/trn_rl_repo")

import numpy as np
import ml_dtypes

import concourse.bacc as bacc
import concourse.bass as bass
import concourse.tile as tile
from concourse import mybir
from concourse.bass_utils import run_bass_kernel_spmd

BF16 = ml_dtypes.bfloat16

B = 2
S = 4096
D = 512
H = 8
DH = 64           # head dim
HPC = 2           # heads per core
D2 = HPC * DH     # 128, the two heads' feature slice
N_CORES = 8
QC = 512          # query chunk (free dim of scores/attnV matmuls)
KVC = 128         # kv chunk (partition dim of transposed scores)
N_QC = S // QC    # 8
N_KVC = S // KVC  # 32
GROUP = 3         # kv chunks per exp() instruction (PSUM banks per S tile)

FP32 = mybir.dt.float32
BF16_T = mybir.dt.bfloat16
AF = mybir.ActivationFunctionType


def build_kernel():
    nc = bacc.Bacc("TRN2", debug=False, enable_asserts=False, num_devices=N_CORES)

    # DRAM I/O (per-core shapes; identical program on every core)
    qT = nc.dram_tensor("qT", [D, S], BF16_T, kind="ExternalInput").ap()
    kT = nc.dram_tensor("kT", [D, S], BF16_T, kind="ExternalInput").ap()
    vT = nc.dram_tensor("vT", [D, S], BF16_T, kind="ExternalInput").ap()
    wqT2 = nc.dram_tensor("wqT2", [D, D2], BF16_T, kind="ExternalInput").ap()
    wkT2 = nc.dram_tensor("wkT2", [D, D2], BF16_T, kind="ExternalInput").ap()
    wvT2 = nc.dram_tensor("wvT2", [D, D2], BF16_T, kind="ExternalInput").ap()
    wo0 = nc.dram_tensor("wo0", [DH, D], BF16_T, kind="ExternalInput").ap()
    wo1 = nc.dram_tensor("wo1", [DH, D], BF16_T, kind="ExternalInput").ap()
    bq2 = nc.dram_tensor("bq2", [D2, 1], FP32, kind="ExternalInput").ap()
    bk2 = nc.dram_tensor("bk2", [D2, 1], FP32, kind="ExternalInput").ap()
    bv2 = nc.dram_tensor("bv2", [1, D2], BF16_T, kind="ExternalInput").ap()
    ouT = nc.dram_tensor("ouT", [D, S], FP32, kind="ExternalOutput").ap()

    KD = D // 128  # 4 contraction chunks of 128

    with tile.TileContext(nc) as tc:
        with (
            tc.tile_pool(name="persist", bufs=1) as pp,
            tc.tile_pool(name="stream", bufs=6) as ps,
            tc.tile_pool(name="ptpool", bufs=4) as ppt,
            tc.tile_pool(name="norm", bufs=3) as pn,
            tc.tile_pool(name="outs", bufs=4) as po,
            tc.tile_pool(name="psum", bufs=1, space="PSUM") as psum,
        ):
            # ---- constants / weights to SBUF ----
            wq_sb = pp.tile([128, KD, D2], BF16_T)
            wk_sb = pp.tile([128, KD, D2], BF16_T)
            wv_sb = pp.tile([128, KD, D2], BF16_T)
            nc.sync.dma_start(out=wq_sb, in_=wqT2.rearrange("(c p) m -> p c m", p=128))
            nc.sync.dma_start(out=wk_sb, in_=wkT2.rearrange("(c p) m -> p c m", p=128))
            nc.sync.dma_start(out=wv_sb, in_=wvT2.rearrange("(c p) m -> p c m", p=128))
            wo_sb = [pp.tile([DH, D], BF16_T, tag=f"wo{h}", name=f"wo{h}") for h in range(HPC)]
            nc.sync.dma_start(out=wo_sb[0], in_=wo0)
            nc.sync.dma_start(out=wo_sb[1], in_=wo1)
            bq_sb = pp.tile([D2, 1], FP32, tag="bq")
            bk_sb = pp.tile([D2, 1], FP32, tag="bk")
            bv_sb = pp.tile([1, D2], BF16_T, tag="bv")
            nc.sync.dma_start(out=bq_sb, in_=bq2)
            nc.sync.dma_start(out=bk_sb, in_=bk2)
            nc.sync.dma_start(out=bv_sb, in_=bv2)
            ones_row = pp.tile([1, 128], BF16_T, tag="ones")
            nc.vector.memset(ones_row, 1.0)

            # ---- persistent activations ----
            qpT = pp.tile([D2, S], BF16_T, tag="qpT")   # [2*dh, s] both heads
            kpT = pp.tile([D2, S], BF16_T, tag="kpT")
            # vp per head: [kv in chunk, chunk, dh+1]; last col = ones (denom)
            vp = [pp.tile([128, N_KVC, 128], BF16_T, tag=f"vp{h}", name=f"vp{h}") for h in range(HPC)]
            for h in range(HPC):
                nc.vector.memset(vp[h][:, :, DH + 1 :], 0.0)
                nc.vector.memset(vp[h][:, :, DH : DH + 1], 1.0)

            # ---- stage A: projections ----
            # qpT / kpT: out[j2, s] = W.T @ xT  (lhsT = w chunk, rhs = xT chunk)
            for name, src_, w_sb, b_sb, dst in (
                ("q", qT, wq_sb, bq_sb, qpT),
                ("k", kT, wk_sb, bk_sb, kpT),
            ):
                xin = [ps.tile([128, S], BF16_T, tag="xin", name=f"xin_{name}_{kc}") for kc in range(KD)]
                for sb in range(4):
                    ss = slice(sb * (S // 4), (sb + 1) * (S // 4))
                    for kc in range(KD):
                        nc.sync.dma_start(
                            out=xin[kc][:, ss], in_=src_[kc * 128 : (kc + 1) * 128, ss]
                        )
                for sc in range(S // 512):
                    pt = psum.tile([D2, 512], FP32, tag=f"s{sc % 2}", name="pt_qk")
                    for kc in range(KD):
                        nc.tensor.matmul(
                            pt,
                            w_sb[:, kc, :],
                            xin[kc][:, sc * 512 : (sc + 1) * 512],
                            start=(kc == 0),
                            stop=(kc == KD - 1),
                        )
                    # evict + per-partition bias on DVE (keeps ACT free for exp)
                    nc.vector.tensor_scalar_add(
                        out=dst[:, sc * 512 : (sc + 1) * 512],
                        in0=pt,
                        scalar1=b_sb,
                    )
            # vp: out[s, j2] = x @ W.T  (lhsT = vT chunk (stationary), rhs = w chunk)
            vin = [ps.tile([128, S], BF16_T, tag="xin", name=f"vin_{kc}") for kc in range(KD)]
            for sb in range(4):
                ss = slice(sb * (S // 4), (sb + 1) * (S // 4))
                for kc in range(KD):
                    nc.sync.dma_start(
                        out=vin[kc][:, ss], in_=vT[kc * 128 : (kc + 1) * 128, ss]
                    )
            for sc in range(N_KVC):
                pt = psum.tile([128, D2], FP32, tag=f"o{sc % 2}", name="pt_v")
                for kc in range(KD):
                    nc.tensor.matmul(
                        pt,
                        vin[kc][:, sc * 128 : (sc + 1) * 128],
                        wv_sb[:, kc, :],
                        start=(kc == 0),
                        stop=False,
                    )
                # bias via rank-1 update: ones[s] x bv[j2]
                nc.tensor.matmul(pt, ones_row, bv_sb, start=False, stop=True)
                for h in range(HPC):
                    nc.vector.tensor_copy(
                        out=vp[h][:, sc, 0:DH], in_=pt[:, h * DH : (h + 1) * DH]
                    )
            # ---- stage B: attention + output projection ----
            # symmetric kv-chunk groups of 3 PSUM banks per head
            # (3+3 score banks + 1+1 attn-out banks = 8; outproj reuses a
            # score slot by qc parity)
            groups = []
            kv = 0
            while kv < N_KVC:
                n = min(GROUP, N_KVC - kv)
                groups.append((kv, n))
                kv += n

            o_tiles = {}

            def make_attn(qc, g0, glen, p_sbs):
                def emit():
                    for gi in range(glen):
                        kvc = g0 + gi
                        for h in range(HPC):
                            nc.tensor.matmul(
                                o_tiles[(qc, h)],
                                vp[h][:, kvc, :],
                                p_sbs[h][:, gi * QC : (gi + 1) * QC],
                                start=(kvc == 0),
                                stop=(kvc == N_KVC - 1),
                            )
                return emit

            def make_norm(qc):
                def emit():
                    ous = []
                    den2 = pn.tile([1, HPC * QC], FP32, tag="den2", name="den2")
                    for h in range(HPC):
                        ou = pn.tile([DH, QC], FP32, tag=f"ou{h}", name=f"ou{h}")
                        nc.vector.tensor_copy(out=ou, in_=o_tiles[(qc, h)][0:DH, :])
                        nc.vector.tensor_copy(
                            out=den2[0:1, h * QC : (h + 1) * QC],
                            in_=o_tiles[(qc, h)][DH : DH + 1, :],
                        )
                        ous.append(ou)
                    rec2 = pn.tile([1, HPC * QC], FP32, tag="rec2", name="rec2")
                    nc.vector.reciprocal_approx_fast(out=rec2, in_=den2)
                    outn = []
                    for h in range(HPC):
                        bcast = pn.tile([DH, QC], FP32, tag=f"bcast{h}", name=f"bcast{h}")
                        nc.gpsimd.partition_broadcast(
                            bcast, rec2[0:1, h * QC : (h + 1) * QC]
                        )
                        on = pn.tile([DH, QC], BF16_T, tag=f"outn{h}", name=f"on{h}")
                        nc.vector.tensor_mul(on, ous[h], bcast)
                        outn.append(on)
                    return outn
                return emit

            def make_proj(qc, outn):
                def emit():
                    qs = slice(qc * QC, (qc + 1) * QC)
                    for ec in range(D // 128):
                        op = psum.tile([128, QC], FP32, tag=f"s{qc % 2}", name="op")
                        nc.tensor.matmul(
                            op, wo_sb[0][:, ec * 128 : (ec + 1) * 128], outn[0],
                            start=True, stop=False,
                        )
                        nc.tensor.matmul(
                            op, wo_sb[1][:, ec * 128 : (ec + 1) * 128], outn[1],
                            start=False, stop=True,
                        )
                        ot = po.tile([128, QC], FP32, tag="ot", name="ot")
                        nc.vector.tensor_copy(out=ot, in_=op)
                        nc.sync.dma_start(
                            out=ouT[ec * 128 : (ec + 1) * 128, qs], in_=ot
                        )
                return emit

            # software pipeline: attnV of each event is deferred one event so
            # the PE never sits behind the exp() it just fed; per-qc epilogue
            # (normalize + outproj) is deferred past that attnV.
            attn_q = []
            pending_norm = None
            pending_proj = None
            for qc in range(N_QC):
                qs = slice(qc * QC, (qc + 1) * QC)
                for h in range(HPC):
                    o_tiles[(qc, h)] = psum.tile(
                        [128, QC], FP32, tag=f"o{h}", name=f"o_ps{h}"
                    )
                for evi, (g0, glen) in enumerate(groups):
                    s_tiles = [
                        psum.tile([128, glen * QC], FP32, tag=f"s{h}", name=f"s_ps{h}")
                        for h in range(HPC)
                    ]
                    for gi in range(glen):
                        kvc = g0 + gi
                        for h in range(HPC):
                            hs = slice(h * DH, (h + 1) * DH)
                            nc.tensor.matmul(
                                s_tiles[h][:, gi * QC : (gi + 1) * QC],
                                kpT[hs, kvc * KVC : (kvc + 1) * KVC],
                                qpT[hs, qs],
                                start=True,
                                stop=True,
                            )
                    p_sbs = []
                    for h in range(HPC):
                        p_sb = ppt.tile(
                            [128, glen * QC], BF16_T, tag=f"pt{h}", name=f"p_sb{h}"
                        )
                        nc.scalar.activation(
                            out=p_sb, in_=s_tiles[h], func=AF.Exp, scale=0.125
                        )
                        p_sbs.append(p_sb)
                    attn_q.append(make_attn(qc, g0, glen, p_sbs))
                    if len(attn_q) > 2:
                        attn_q.pop(0)()
                    if pending_norm is not None:
                        outn = pending_norm()
                        pending_proj = make_proj(qc - 1, outn)
                        pending_norm = None
                    elif pending_proj is not None and evi >= 6:
                        pending_proj()
                        pending_proj = None
                while attn_q:
                    attn_q.pop(0)()
                pending_norm = make_norm(qc)
            outn = pending_norm()
            make_proj(N_QC - 1, outn)()
    nc.compile()
    return nc


_NC_CACHE = None


def _get_nc():
    global _NC_CACHE
    if _NC_CACHE is None:
        _NC_CACHE = build_kernel()
    return _NC_CACHE


def make_in_maps(q, k, v, w_q, b_q, w_k, b_k, w_v, b_v, w_o, b_o):
    """Shard the full inputs into the 8 per-core input maps."""
    q = np.asarray(q, np.float32)
    k = np.asarray(k, np.float32)
    v = np.asarray(v, np.float32)
    w_q = np.asarray(w_q, np.float32)
    w_k = np.asarray(w_k, np.float32)
    w_v = np.asarray(w_v, np.float32)
    w_o = np.asarray(w_o, np.float32)
    b_q = np.asarray(b_q, np.float32)
    b_k = np.asarray(b_k, np.float32)
    b_v = np.asarray(b_v, np.float32)

    qT = [np.ascontiguousarray(q[b].T).astype(BF16) for b in range(B)]
    kTb = [np.ascontiguousarray(k[b].T).astype(BF16) for b in range(B)]
    vTb = [np.ascontiguousarray(v[b].T).astype(BF16) for b in range(B)]
    wqT = np.ascontiguousarray(w_q.T).astype(BF16)  # [D, D] = [d, j]
    wkT = np.ascontiguousarray(w_k.T).astype(BF16)
    wvT = np.ascontiguousarray(w_v.T).astype(BF16)

    in_maps = []
    for c in range(N_CORES):
        b = c // 4
        hp = c % 4
        js = slice(hp * D2, (hp + 1) * D2)
        h0 = hp * D2
        in_maps.append(
            {
                "qT": qT[b],
                "kT": kTb[b],
                "vT": vTb[b],
                "wqT2": np.ascontiguousarray(wqT[:, js]),
                "wkT2": np.ascontiguousarray(wkT[:, js]),
                "wvT2": np.ascontiguousarray(wvT[:, js]),
                "wo0": np.ascontiguousarray(w_o[:, h0 : h0 + DH].T).astype(BF16),
                "wo1": np.ascontiguousarray(w_o[:, h0 + DH : h0 + 2 * DH].T).astype(BF16),
                "bq2": np.ascontiguousarray(b_q[js].reshape(D2, 1)),
                "bk2": np.ascontiguousarray(b_k[js].reshape(D2, 1)),
                "bv2": np.ascontiguousarray(b_v[js].reshape(1, D2)).astype(BF16),
            }
        )
    return in_maps


def gather_output(results, b_o):
    """Sum per-batch partials, add output bias, restore [B, S, D] layout."""
    b_o = np.asarray(b_o, np.float32)
    out = np.empty((B, S, D), np.float32)
    for b in range(B):
        acc = np.zeros((D, S), np.float32)
        for c in range(b * 4, b * 4 + 4):
            acc += results[c]["ouT"]
        out[b] = acc.T + b_o[None, :]
    return out


def kernel(q, k, v, w_q, b_q, w_k, b_k, w_v, b_v, w_o, b_o):
    nc = _get_nc()
    in_maps = make_in_maps(q, k, v, w_q, b_q, w_k, b_k, w_v, b_v, w_o, b_o)
    res = run_bass_kernel_spmd(nc, in_maps, core_ids=list(range(N_CORES)))
    return gather_output(res.results, b_o)


# revision 20
# speedup vs baseline: 1.1770x; 1.0108x over previous
"""Multi-head attention (B=2, S=4096, D=512, H=8) on 8 Trainium2 NeuronCores.

Sharding: batch x head-pair.  Core c handles batch b = c//4 and heads
(2*(c%4), 2*(c%4)+1).  Each core computes its heads' Q/K/V projections,
flash-style attention (scores kept transposed [kv, q] so the attn@V matmul
consumes the exp() output directly, with softmax denominators accumulated via
an extra ones-column on V), and its heads' slice of the output projection.
The 4 per-batch partial outputs are summed on the host (row-parallel linear)
and the output bias is added there.

All matmul operands are bf16 (f32 PSUM accumulation); inputs are transposed
and cast on the host so the device consumes [d, s]-layout activations
directly (no on-device transposes).
"""

import sys

sys.path.insert(0, "/opt/trn_rl_repo")

import numpy as np
import ml_dtypes

import concourse.bacc as bacc
import concourse.bass as bass
import concourse.tile as tile
from concourse import mybir
from concourse.bass_utils import run_bass_kernel_spmd

BF16 = ml_dtypes.bfloat16

B = 2
S = 4096
D = 512
H = 8
DH = 64           # head dim
HPC = 2           # heads per core
D2 = HPC * DH     # 128, the two heads' feature slice
N_CORES = 8
QC = 512          # query chunk (free dim of scores/attnV matmuls)
KVC = 128         # kv chunk (partition dim of transposed scores)
N_QC = S // QC    # 8
N_KVC = S // KVC  # 32
GROUP = 3         # kv chunks per exp() instruction (PSUM banks per S tile)

FP32 = mybir.dt.float32
BF16_T = mybir.dt.bfloat16
AF = mybir.ActivationFunctionType


def build_kernel():
    nc = bacc.Bacc("TRN2", debug=False, enable_asserts=False, num_devices=N_CORES)

    # DRAM I/O (per-core shapes; identical program on every core)
    qT = nc.dram_tensor("qT", [D, S], BF16_T, kind="ExternalInput").ap()
    kT = nc.dram_tensor("kT", [D, S], BF16_T, kind="ExternalInput").ap()
    vT = nc.dram_tensor("vT", [D, S], BF16_T, kind="ExternalInput").ap()
    wqT2 = nc.dram_tensor("wqT2", [D, D2], BF16_T, kind="ExternalInput").ap()
    wkT2 = nc.dram_tensor("wkT2", [D, D2], BF16_T, kind="ExternalInput").ap()
    wvT2 = nc.dram_tensor("wvT2", [D, D2], BF16_T, kind="ExternalInput").ap()
    wo0 = nc.dram_tensor("wo0", [DH, D], BF16_T, kind="ExternalInput").ap()
    wo1 = nc.dram_tensor("wo1", [DH, D], BF16_T, kind="ExternalInput").ap()
    bq2 = nc.dram_tensor("bq2", [D2, 1], FP32, kind="ExternalInput").ap()
    bk2 = nc.dram_tensor("bk2", [D2, 1], FP32, kind="ExternalInput").ap()
    bv2 = nc.dram_tensor("bv2", [1, D2], BF16_T, kind="ExternalInput").ap()
    ouT = nc.dram_tensor("ouT", [D, S], FP32, kind="ExternalOutput").ap()

    KD = D // 128  # 4 contraction chunks of 128

    with tile.TileContext(nc) as tc:
        with (
            tc.tile_pool(name="persist", bufs=1) as pp,
            tc.tile_pool(name="stream", bufs=6) as ps,
            tc.tile_pool(name="ptpool", bufs=4) as ppt,
            tc.tile_pool(name="norm", bufs=3) as pn,
            tc.tile_pool(name="outs", bufs=4) as po,
            tc.tile_pool(name="psum", bufs=1, space="PSUM") as psum,
        ):
            # ---- constants / weights to SBUF ----
            wq_sb = pp.tile([128, KD, D2], BF16_T)
            wk_sb = pp.tile([128, KD, D2], BF16_T)
            wv_sb = pp.tile([128, KD, D2], BF16_T)
            nc.sync.dma_start(out=wq_sb, in_=wqT2.rearrange("(c p) m -> p c m", p=128))
            nc.sync.dma_start(out=wk_sb, in_=wkT2.rearrange("(c p) m -> p c m", p=128))
            nc.sync.dma_start(out=wv_sb, in_=wvT2.rearrange("(c p) m -> p c m", p=128))
            wo_sb = [pp.tile([DH, D], BF16_T, tag=f"wo{h}", name=f"wo{h}") for h in range(HPC)]
            nc.sync.dma_start(out=wo_sb[0], in_=wo0)
            nc.sync.dma_start(out=wo_sb[1], in_=wo1)
            bq_sb = pp.tile([D2, 1], FP32, tag="bq")
            bk_sb = pp.tile([D2, 1], FP32, tag="bk")
            bv_sb = pp.tile([1, D2], BF16_T, tag="bv")
            nc.sync.dma_start(out=bq_sb, in_=bq2)
            nc.sync.dma_start(out=bk_sb, in_=bk2)
            nc.sync.dma_start(out=bv_sb, in_=bv2)
            ones_row = pp.tile([1, 128], BF16_T, tag="ones")
            nc.vector.memset(ones_row, 1.0)

            # ---- persistent activations ----
            qpT = pp.tile([D2, S], BF16_T, tag="qpT")   # [2*dh, s] both heads
            kpT = pp.tile([D2, S], BF16_T, tag="kpT")
            # vp per head: [kv in chunk, chunk, dh+1]; last col = ones (denom)
            vp = [pp.tile([128, N_KVC, 128], BF16_T, tag=f"vp{h}", name=f"vp{h}") for h in range(HPC)]
            for h in range(HPC):
                nc.vector.memset(vp[h][:, :, DH + 1 :], 0.0)
                nc.vector.memset(vp[h][:, :, DH : DH + 1], 1.0)

            # ---- stage A: projections ----
            # qpT / kpT: out[j2, s] = W.T @ xT  (lhsT = w chunk, rhs = xT chunk)
            for name, src_, w_sb, b_sb, dst in (
                ("q", qT, wq_sb, bq_sb, qpT),
                ("k", kT, wk_sb, bk_sb, kpT),
            ):
                xin = [ps.tile([128, S], BF16_T, tag="xin", name=f"xin_{name}_{kc}") for kc in range(KD)]
                for sb in range(4):
                    ss = slice(sb * (S // 4), (sb + 1) * (S // 4))
                    for kc in range(KD):
                        nc.sync.dma_start(
                            out=xin[kc][:, ss], in_=src_[kc * 128 : (kc + 1) * 128, ss]
                        )
                for sc in range(S // 512):
                    pt = psum.tile([D2, 512], FP32, tag=f"s{sc % 2}", name="pt_qk")
                    for kc in range(KD):
                        nc.tensor.matmul(
                            pt,
                            w_sb[:, kc, :],
                            xin[kc][:, sc * 512 : (sc + 1) * 512],
                            start=(kc == 0),
                            stop=(kc == KD - 1),
                        )
                    # evict + per-partition bias on DVE (keeps ACT free for exp)
                    nc.vector.tensor_scalar_add(
                        out=dst[:, sc * 512 : (sc + 1) * 512],
                        in0=pt,
                        scalar1=b_sb,
                    )
            # vp: out[s, j2] = x @ W.T  (lhsT = vT chunk (stationary), rhs = w chunk)
            vin = [ps.tile([128, S], BF16_T, tag="xin", name=f"vin_{kc}") for kc in range(KD)]
            for sb in range(4):
                ss = slice(sb * (S // 4), (sb + 1) * (S // 4))
                for kc in range(KD):
                    nc.sync.dma_start(
                        out=vin[kc][:, ss], in_=vT[kc * 128 : (kc + 1) * 128, ss]
                    )
            for sc in range(N_KVC):
                pt = psum.tile([128, D2], FP32, tag=f"o{sc % 2}", name="pt_v")
                for kc in range(KD):
                    nc.tensor.matmul(
                        pt,
                        vin[kc][:, sc * 128 : (sc + 1) * 128],
                        wv_sb[:, kc, :],
                        start=(kc == 0),
                        stop=False,
                    )
                # bias via rank-1 update: ones[s] x bv[j2]
                nc.tensor.matmul(pt, ones_row, bv_sb, start=False, stop=True)
                for h in range(HPC):
                    nc.vector.tensor_copy(
                        out=vp[h][:, sc, 0:DH], in_=pt[:, h * DH : (h + 1) * DH]
                    )
            # ---- stage B: attention + output projection ----
            # symmetric kv-chunk groups of 3 PSUM banks per head
            # (3+3 score banks + 1+1 attn-out banks = 8; outproj reuses a
            # score slot by qc parity)
            groups = []
            kv = 0
            while kv < N_KVC:
                n = min(GROUP, N_KVC - kv)
                groups.append((kv, n))
                kv += n

            o_tiles = {}

            def make_attn(qc, g0, glen, p_sbs):
                def emit():
                    for gi in range(glen):
                        kvc = g0 + gi
                        for h in range(HPC):
                            nc.tensor.matmul(
                                o_tiles[(qc, h)],
                                vp[h][:, kvc, :],
                                p_sbs[h][:, gi * QC : (gi + 1) * QC],
                                start=(kvc == 0),
                                stop=(kvc == N_KVC - 1),
                            )
                return emit

            def make_norm(qc):
                def emit():
                    ous = []
                    den2 = pn.tile([1, HPC * QC], FP32, tag="den2", name="den2")
                    for h in range(HPC):
                        ou = pn.tile([DH, QC], FP32, tag=f"ou{h}", name=f"ou{h}")
                        nc.vector.tensor_copy(out=ou, in_=o_tiles[(qc, h)][0:DH, :])
                        nc.vector.tensor_copy(
                            out=den2[0:1, h * QC : (h + 1) * QC],
                            in_=o_tiles[(qc, h)][DH : DH + 1, :],
                        )
                        ous.append(ou)
                    rec2 = pn.tile([1, HPC * QC], FP32, tag="rec2", name="rec2")
                    nc.vector.reciprocal_approx_fast(out=rec2, in_=den2)
                    outn = []
                    for h in range(HPC):
                        bcast = pn.tile([DH, QC], FP32, tag=f"bcast{h}", name=f"bcast{h}")
                        nc.gpsimd.partition_broadcast(
                            bcast, rec2[0:1, h * QC : (h + 1) * QC]
                        )
                        on = pn.tile([DH, QC], BF16_T, tag=f"outn{h}", name=f"on{h}")
                        nc.vector.tensor_mul(on, ous[h], bcast)
                        outn.append(on)
                    return outn
                return emit

            def make_proj(qc, outn):
                def emit():
                    qs = slice(qc * QC, (qc + 1) * QC)
                    for ec in range(D // 128):
                        op = psum.tile([128, QC], FP32, tag=f"s{qc % 2}", name="op")
                        nc.tensor.matmul(
                            op, wo_sb[0][:, ec * 128 : (ec + 1) * 128], outn[0],
                            start=True, stop=False,
                        )
                        nc.tensor.matmul(
                            op, wo_sb[1][:, ec * 128 : (ec + 1) * 128], outn[1],
                            start=False, stop=True,
                        )
                        ot = po.tile([128, QC], FP32, tag="ot", name="ot")
                        nc.vector.tensor_copy(out=ot, in_=op)
                        nc.sync.dma_start(
                            out=ouT[ec * 128 : (ec + 1) * 128, qs], in_=ot
                        )
                return emit

            # software pipeline: attnV of each event is deferred one event so
            # the PE never sits behind the exp() it just fed; per-qc epilogue
            # (normalize + outproj) is deferred past that attnV.
            attn_q = []      # (qc, emit_fn, is_last_group_of_qc)
            norm_out = {}    # qc -> outn tiles
            proj_cd = None   # (countdown, qc)
            LAG = 2

            def pump(drain=False):
                nonlocal proj_cd
                while len(attn_q) > (0 if drain else LAG):
                    aqc, fn, last = attn_q.pop(0)
                    fn()
                    if last:
                        norm_out[aqc] = make_norm(aqc)()
                        proj_cd = [4, aqc]
                if proj_cd is not None:
                    if drain:
                        proj_cd[0] = 0
                    if proj_cd[0] <= 0:
                        pqc = proj_cd[1]
                        make_proj(pqc, norm_out.pop(pqc))()
                        proj_cd = None
                    else:
                        proj_cd[0] -= 1

            for qc in range(N_QC):
                qs = slice(qc * QC, (qc + 1) * QC)
                for h in range(HPC):
                    o_tiles[(qc, h)] = psum.tile(
                        [128, QC], FP32, tag=f"o{h}", name=f"o_ps{h}"
                    )
                for evi, (g0, glen) in enumerate(groups):
                    s_tiles = [
                        psum.tile([128, glen * QC], FP32, tag=f"s{h}", name=f"s_ps{h}")
                        for h in range(HPC)
                    ]
                    for gi in range(glen):
                        kvc = g0 + gi
                        for h in range(HPC):
                            hs = slice(h * DH, (h + 1) * DH)
                            nc.tensor.matmul(
                                s_tiles[h][:, gi * QC : (gi + 1) * QC],
                                kpT[hs, kvc * KVC : (kvc + 1) * KVC],
                                qpT[hs, qs],
                                start=True,
                                stop=True,
                            )
                    p_sbs = []
                    for h in range(HPC):
                        p_sb = ppt.tile(
                            [128, glen * QC], BF16_T, tag=f"pt{h}", name=f"p_sb{h}"
                        )
                        nc.scalar.activation(
                            out=p_sb, in_=s_tiles[h], func=AF.Exp, scale=0.125
                        )
                        p_sbs.append(p_sb)
                    attn_q.append(
                        (qc, make_attn(qc, g0, glen, p_sbs), g0 + glen == N_KVC)
                    )
                    pump()
            pump(drain=True)
    nc.compile()
    return nc


_NC_CACHE = None


def _get_nc():
    global _NC_CACHE
    if _NC_CACHE is None:
        _NC_CACHE = build_kernel()
    return _NC_CACHE


def make_in_maps(q, k, v, w_q, b_q, w_k, b_k, w_v, b_v, w_o, b_o):
    """Shard the full inputs into the 8 per-core input maps."""
    q = np.asarray(q, np.float32)
    k = np.asarray(k, np.float32)
    v = np.asarray(v, np.float32)
    w_q = np.asarray(w_q, np.float32)
    w_k = np.asarray(w_k, np.float32)
    w_v = np.asarray(w_v, np.float32)
    w_o = np.asarray(w_o, np.float32)
    b_q = np.asarray(b_q, np.float32)
    b_k = np.asarray(b_k, np.float32)
    b_v = np.asarray(b_v, np.float32)

    qT = [np.ascontiguousarray(q[b].T).astype(BF16) for b in range(B)]
    kTb = [np.ascontiguousarray(k[b].T).astype(BF16) for b in range(B)]
    vTb = [np.ascontiguousarray(v[b].T).astype(BF16) for b in range(B)]
    wqT = np.ascontiguousarray(w_q.T).astype(BF16)  # [D, D] = [d, j]
    wkT = np.ascontiguousarray(w_k.T).astype(BF16)
    wvT = np.ascontiguousarray(w_v.T).astype(BF16)

    in_maps = []
    for c in range(N_CORES):
        b = c // 4
        hp = c % 4
        js = slice(hp * D2, (hp + 1) * D2)
        h0 = hp * D2
        in_maps.append(
            {
                "qT": qT[b],
                "kT": kTb[b],
                "vT": vTb[b],
                "wqT2": np.ascontiguousarray(wqT[:, js]),
                "wkT2": np.ascontiguousarray(wkT[:, js]),
                "wvT2": np.ascontiguousarray(wvT[:, js]),
                "wo0": np.ascontiguousarray(w_o[:, h0 : h0 + DH].T).astype(BF16),
                "wo1": np.ascontiguousarray(w_o[:, h0 + DH : h0 + 2 * DH].T).astype(BF16),
                "bq2": np.ascontiguousarray(b_q[js].reshape(D2, 1)),
                "bk2": np.ascontiguousarray(b_k[js].reshape(D2, 1)),
                "bv2": np.ascontiguousarray(b_v[js].reshape(1, D2)).astype(BF16),
            }
        )
    return in_maps


def gather_output(results, b_o):
    """Sum per-batch partials, add output bias, restore [B, S, D] layout."""
    b_o = np.asarray(b_o, np.float32)
    out = np.empty((B, S, D), np.float32)
    for b in range(B):
        acc = np.zeros((D, S), np.float32)
        for c in range(b * 4, b * 4 + 4):
            acc += results[c]["ouT"]
        out[b] = acc.T + b_o[None, :]
    return out


def kernel(q, k, v, w_q, b_q, w_k, b_k, w_v, b_v, w_o, b_o):
    nc = _get_nc()
    in_maps = make_in_maps(q, k, v, w_q, b_q, w_k, b_k, w_v, b_v, w_o, b_o)
    res = run_bass_kernel_spmd(nc, in_maps, core_ids=list(range(N_CORES)))
    return gather_output(res.results, b_o)
